# revision 19
# baseline (speedup 1.0000x reference)
"""Trainium2 Bass kernel for the CMPO2/GTN MPS-chain contraction (v6).

Computation (see harness reference): for each sample s,
    v0  = psi_first^T x[s,0]                                  [D]
    v_{i+1}[e] = sum_{d,p} v_i[d] psi_mid[i][d,e,p] x[s,1+i,p]   (62 steps)
    out_vec[s] = sum_{d,p} v[d] psi_last[d,p,o] x[s,63,p]     [B, O]
    out[s] = c * out_vec[s]   with c the (batch-independent) phi-chain scalar.

Mapping: batch-parallel over 8 cores (1024 samples/core).  Each middle step is
    u[(p,d), s] = v[d, s] * x[s, p]     (DVE fp16 2x elementwise)
    v_new       = A_flat^T @ u          (PE, K=2048 in 16 chunks of 128)
with the [vT; vT] stationary-duplication trick giving the 128-row v operand
for free from PSUM.

v6 structure (why it is shaped this way):
  * The per-step serial chain (PSUM evac -> v2 -> DVE muls -> matmuls) is
    ~12us and cannot be overlapped within one chain.  The batch is split into
    TWO independent half-chains (512 samples each) interleaved half a step
    out of phase, so the kernel is engine-throughput-bound (~10us/step, the
    DVE fp16 2x multiply floor) instead of chain-latency-bound.
  * 6 of 16 x-broadcast chunks are produced on-chip by exact 0/1-stationary
    replication matmuls (K=32, row-tiled on distinct PE array quadrants so
    pair-tiles run concurrently), the other 10 streamed pre-broadcast from
    HBM.  Cuts HBM from ~4.5MB/step to ~3.3MB/step, below the DVE floor.
  * Broadcast matmuls + their PSUM evacs are interleaved INTO the main
    matmul stream at three points (spaced so PSUM slot recycling never
    blocks the in-order PE queue) and produce tiles for step i+1, keeping
    them entirely off the critical chain.
  * phi chain scalar is computed on the host (batch-independent) and folded
    into the final output scale; everything streams prefetched 2 steps out.
"""

import numpy as np

N_CORES = 8
B, Q, P, D, L, O = 8192, 64, 32, 64, 64, 10
BL = B // N_CORES          # batch per core
TN = 512                   # matmul free-dim tile == half-batch
NCH = (D * P) // 128       # 16 K-chunks of 128 over (p,d)
NMID = L - 2               # 62 middle sites
SH_LAST = 6                # 2^SH_LAST folded into psi_last
VBAND = 16.0               # target |v| band for the scale schedule

NB = 6                     # chunks 10..15 produced via PE broadcast
NST = NCH - NB             # 10 streamed chunks
ROW_TILED = True

# global row r in 0..2047 of u/(A rows): p = 2*(r//128) + (r%128)//64 ; d = r%64
_P_IDX = np.repeat(np.arange(P), D)          # [2048]
_D_IDX = np.tile(np.arange(D), P)            # [2048]

_cached = {}


def _ensure_path():
    import sys
    for p in ("/opt/trn_rl_repo", "/root/.axon_site/_ro/trn_rl_repo"):
        try:
            import concourse  # noqa: F401
            return
        except Exception:
            if p not in sys.path:
                sys.path.insert(0, p)
    import concourse  # noqa: F401


def _build_program():
    """Build + compile the Bass/Tile program (shared by all 8 cores)."""
    _ensure_path()
    from concourse import bacc, tile, mybir

    dt = mybir.dt
    nc = bacc.Bacc(
        "TRN2",
        target_bir_lowering=False,
        debug=False,
        enable_asserts=False,
        num_devices=N_CORES,
    )

    # xbm: streamed chunks 0..7, layout [i, r, (h2, g2, c4, s)]
    # xg2: streamed part of the mixed group (chunks 8, 9): [i, r, (h2, c2, s)]
    a_d = nc.dram_tensor("a_w", [NMID, 128, NCH * 128], dt.float16, kind="ExternalInput").ap()
    xbm_d = nc.dram_tensor("xbm", [NMID + 1, 128, 16 * TN], dt.float16, kind="ExternalInput").ap()
    xg2_d = nc.dram_tensor("xg2", [NMID + 1, 128, 4 * TN], dt.float16, kind="ExternalInput").ap()
    x4_d = nc.dram_tensor("x4", [NMID + 1, 128, BL], dt.float16, kind="ExternalInput").ap()
    rep_d = nc.dram_tensor("rep", [128, 2 * 128], dt.float16, kind="ExternalInput").ap()
    x0_d = nc.dram_tensor("x0", [P, BL], dt.float16, kind="ExternalInput").ap()
    pf_d = nc.dram_tensor("pf", [P, 128], dt.float16, kind="ExternalInput").ap()
    pl_d = nc.dram_tensor("pl", [128, NCH * O], dt.float16, kind="ExternalInput").ap()
    sc_d = nc.dram_tensor("fscale", [1, 1], dt.float32, kind="ExternalInput").ap()
    out_d = nc.dram_tensor("out", [O, BL], dt.float32, kind="ExternalOutput").ap()

    with tile.TileContext(nc) as tc:
        with tc.tile_pool(name="const", bufs=1) as cpool, \
             tc.tile_pool(name="aw", bufs=3) as apool, \
             tc.tile_pool(name="xbp", bufs=2) as xbpool, \
             tc.tile_pool(name="bcg", bufs=8) as bcgpool, \
             tc.tile_pool(name="x4p", bufs=3) as x4pool, \
             tc.tile_pool(name="vrp", bufs=4) as vrpool, \
             tc.tile_pool(name="up", bufs=10) as upool, \
             tc.tile_pool(name="misc", bufs=1) as mpool, \
             tc.tile_pool(name="pvp", bufs=4, space="PSUM") as pvpool, \
             tc.tile_pool(name="bcp", bufs=2, space="PSUM") as bcpool:

            pf_sb = cpool.tile([P, 128], dt.float16, name="pf_sb")
            nc.sync.dma_start(out=pf_sb, in_=pf_d)
            pl_sb = cpool.tile([128, NCH * O], dt.float16, name="pl_sb")
            nc.sync.dma_start(out=pl_sb, in_=pl_d)
            x0_sb = cpool.tile([P, BL], dt.float16, name="x0_sb")
            nc.sync.dma_start(out=x0_sb, in_=x0_d)
            rep_sb = cpool.tile([128, 2 * 128], dt.float16, name="rep_sb")
            nc.sync.dma_start(out=rep_sb, in_=rep_d)
            sc_sb = mpool.tile([1, 1], dt.float32, name="sc_sb")
            nc.sync.dma_start(out=sc_sb, in_=sc_d)
            sc10 = mpool.tile([O, 1], dt.float32, name="sc10")
            nc.gpsimd.partition_broadcast(sc10, sc_sb)

            def emit_loads(i):
                """Prefetch step i's streamed tiles; returns
                (xbm, [bcg2_h0, bcg2_h1, bcg3_h0, bcg3_h1], x4, a_raw)."""
                xbm = xbpool.tile([128, 16 * TN], dt.float16, name="xbm")
                half = 8 * TN
                nc.sync.dma_start(out=xbm[:, :half], in_=xbm_d[i, :, :half])
                nc.scalar.dma_start(out=xbm[:, half:], in_=xbm_d[i, :, half:])
                bcg = []
                for h in range(2):
                    g2 = bcgpool.tile([128, 4 * TN], dt.float16, name="bcg2")
                    eng = nc.sync if h == 0 else nc.scalar
                    eng.dma_start(out=g2[:, 0:2 * TN],
                                  in_=xg2_d[i, :, h * 2 * TN:(h + 1) * 2 * TN])
                    bcg.append(g2)
                for h in range(2):
                    g3 = bcgpool.tile([128, 4 * TN], dt.float16, name="bcg3")
                    bcg.append(g3)
                x4_sb = x4pool.tile([128, BL], dt.float16, name="x4_sb")
                nc.sync.dma_start(out=x4_sb, in_=x4_d[i])
                a_raw = None
                if i < NMID:
                    a_raw = apool.tile([128, NCH * 128], dt.float16, name="a_raw")
                    nc.scalar.dma_start(out=a_raw, in_=a_d[i])
                return xbm, bcg, x4_sb, a_raw

            def emit_bcast_pair(jp, x4_sb, bcg, h):
                """PSUM pair-tile for bcast chunks (10+2jp, 11+2jp), half h:
                two row-tiled matmuls on distinct quadrants, one FD-1024
                scalar evac into the bcg tile columns."""
                bps = bcpool.tile([128, 2 * TN], dt.float32, name="bps")
                for u in range(2):
                    j = 2 * jp + u
                    k = (j % 4) if ROW_TILED else 0
                    jq = (j // 4) if ROW_TILED else 0
                    nc.tensor.matmul(
                        out=bps[:, u * TN:(u + 1) * TN],
                        lhsT=rep_sb[32 * k:32 * (k + 1), jq * 128:(jq + 1) * 128],
                        rhs=x4_sb[32 * k:32 * (k + 1), h * TN:(h + 1) * TN],
                        start=True, stop=True,
                        tile_position=(32 * k, 0) if ROW_TILED else None)
                dest = bcg[h] if jp == 0 else bcg[2 + h]
                off = 2 * TN if jp in (0, 2) else 0
                nc.scalar.copy(out=dest[:, off:off + 2 * TN], in_=bps)

            # --- v0 per half ---
            pv_cur = []
            for h in range(2):
                pv = pvpool.tile([128, TN], dt.float32, name="pv")
                nc.tensor.matmul(out=pv, lhsT=pf_sb,
                                 rhs=x0_sb[:, h * TN:(h + 1) * TN],
                                 start=True, stop=True)
                pv_cur.append(pv)

            tiles_cur = emit_loads(0)
            for h in range(2):                  # boot: bcast tiles for step 0
                for jp in range(3):
                    emit_bcast_pair(jp, tiles_cur[2], tiles_cur[1], h)
            tiles_nxt = emit_loads(1)

            for i in range(NMID + 1):
                last = (i == NMID)
                xbm, bcg, x4_sb, a_raw = tiles_cur
                a_lhs = None if last else a_raw.rearrange("p (c m) -> p c m", c=NCH)

                pv_nxt = []
                for h in range(2):
                    pv = pvpool.tile([O if last else 128, TN], dt.float32, name="pv")
                    pv_nxt.append(pv)

                tiles_fut = None
                for h in range(2):
                    # v4 = v duplicated over 4 column blocks (ACT only)
                    v4 = vrpool.tile([128, 4 * TN], dt.float16, name="v4")
                    nc.scalar.copy(out=v4[:, 0:TN], in_=pv_cur[h])
                    nc.scalar.copy(out=v4[:, TN:2 * TN], in_=v4[:, 0:TN])
                    nc.scalar.copy(out=v4[:, 2 * TN:4 * TN], in_=v4[:, 0:2 * TN])

                    # u products: 4 muls of [128, 4*TN] fp16 (DVE 2x)
                    us = []
                    for g in range(4):
                        u4 = upool.tile([128, 4 * TN], dt.float16, name="u4")
                        if g < 2:
                            xin = xbm[:, (h * 2 + g) * 4 * TN:(h * 2 + g + 1) * 4 * TN]
                        else:
                            xin = bcg[(g - 2) * 2 + h]
                        nc.vector.tensor_mul(u4, v4, xin)
                        us.append(u4)

                    if h == 0 and i + 2 <= NMID:
                        tiles_fut = emit_loads(i + 2)

                    # main matmuls, chunk-outer; next step's bcast pair-tiles
                    # interleave into half-0's stream (PSUM slots recycle
                    # across >=4 main chunks so the PE never blocks)
                    for c in range(NCH):
                        g, cc = c // 4, c % 4
                        lhs = pl_sb[:, c * O:(c + 1) * O] if last else a_lhs[:, c]
                        nc.tensor.matmul(
                            out=pv_nxt[h],
                            lhsT=lhs,
                            rhs=us[g][:, cc * TN:(cc + 1) * TN],
                            start=(c == 0), stop=(c == NCH - 1))
                        if not last and h == 0 and c in (5, 10, 15):
                            jp = {5: 0, 10: 1, 15: 2}[c]
                            emit_bcast_pair(jp, tiles_nxt[2], tiles_nxt[1], 0)
                            emit_bcast_pair(jp, tiles_nxt[2], tiles_nxt[1], 1)

                pv_cur = pv_nxt
                if not last:
                    tiles_cur = tiles_nxt
                    tiles_nxt = tiles_fut

            # --- final scale by host-folded phi scalar ---
            out_sb = mpool.tile([O, BL], dt.float32, name="out_sb")
            for h in range(2):
                nc.scalar.mul(out=out_sb[:, h * TN:(h + 1) * TN],
                              in_=pv_cur[h], mul=sc10)
            nc.sync.dma_start(out=out_d, in_=out_sb)

    nc.compile()
    return nc


def _scale_schedule(x, psi_first, psi_mid, nsub=128):
    """Static per-step power-of-2 downscales keeping |v| in a small band."""
    xs = np.asarray(x[:nsub], np.float32)
    v = xs[:, 0] @ np.asarray(psi_first, np.float32).T
    ks = []
    for i in range(NMID):
        A = np.asarray(psi_mid[i], np.float32)            # [d, e, p]
        xi = xs[:, 1 + i]                                  # [s, p]
        u = np.einsum('sd,sp->sdp', v, xi).reshape(nsub, D * P)
        v = u @ A.transpose(0, 2, 1).reshape(D * P, D)
        vm = float(np.abs(v).max())
        k = 0
        while vm * 2.0 ** (-k) > VBAND:
            k += 1
        ks.append(k)
        v = v * 2.0 ** (-k)
    return ks


def kernel(x, psi_first, psi_mid, psi_last, phi_first, phi_mid, phi_last):
    _ensure_path()
    from concourse import bass_utils

    f16 = np.float16
    x = np.asarray(x, np.float32)
    psi_first = np.asarray(psi_first, np.float32)
    psi_mid = np.asarray(psi_mid, np.float32)
    psi_last = np.asarray(psi_last, np.float32)
    phi_first = np.asarray(phi_first, np.float64)
    phi_mid = np.asarray(phi_mid, np.float64)
    phi_last = np.asarray(phi_last, np.float64)

    if "nc" not in _cached:
        _cached["nc"] = _build_program()
    nc = _cached["nc"]

    ks = _scale_schedule(x, psi_first, psi_mid)

    # host phi chain: batch-independent scalar c
    w = phi_first[:, 0].copy()
    for i in range(NMID):
        w = w @ phi_mid[i, :, :, 1 + i]
    c_phi = float(w @ phi_last[:, Q - 1])

    scales = (2.0 ** -np.asarray(ks, np.float64)).astype(np.float32)
    # A2[i, r, e] = psi_mid[i, d(r), e, p(r)] * s_i  -> [62, 2048, 64]
    A2 = psi_mid.transpose(0, 1, 3, 2)[:, _D_IDX, _P_IDX, :]
    A2 = A2 * scales[:, None, None]
    A2c = A2.reshape(NMID, NCH, 128, D)
    # pre-duplicated stationary columns: [i, r, c*128 + j*64 + e]
    A2d = np.broadcast_to(A2c[:, :, :, None, :], (NMID, NCH, 128, 2, D))
    a_host = np.ascontiguousarray(
        A2d.transpose(0, 2, 1, 3, 4).reshape(NMID, 128, NCH * 128)
    ).astype(f16)

    pf_host = np.concatenate([psi_first.T, psi_first.T], axis=1).astype(f16)

    pl2 = (psi_last * (2.0 ** SH_LAST))[_D_IDX, _P_IDX, :]
    pl_host = np.ascontiguousarray(
        pl2.reshape(NCH, 128, O).transpose(1, 0, 2).reshape(128, NCH * O)
    ).astype(f16)

    fscale_host = np.array(
        [[c_phi * 2.0 ** (sum(ks) - SH_LAST)]], dtype=np.float32)

    # rep[32k + p, jq*128 + m] = 1 if p == 2*c + m//64 (c = 10 + j)
    rep_host = np.zeros((128, 2 * 128), f16)
    for j in range(NB):
        c = NST + j
        k = (j % 4) if ROW_TILED else 0
        jq = (j // 4) if ROW_TILED else 0
        for m in range(128):
            p = 2 * c + m // 64
            rep_host[32 * k + p, jq * 128 + m] = 1.0

    # per-core batch shards
    xt = x.transpose(1, 2, 0).astype(f16)         # [Q, P, B]
    x0_all = xt[0]
    ridx = np.arange(128) // 64
    in_maps = []
    for ci in range(N_CORES):
        sl = slice(ci * BL, (ci + 1) * BL)
        xs = np.ascontiguousarray(xt[1:, :, sl])            # [63, P, BL]
        xbm = np.empty((NMID + 1, 128, 2, 2, 4, TN), f16)
        xg2 = np.empty((NMID + 1, 128, 2, 2, TN), f16)
        for h in range(2):
            scol = slice(h * TN, (h + 1) * TN)
            for g in range(2):
                for c4 in range(4):
                    xbm[:, :, h, g, c4, :] = xs[:, 2 * (4 * g + c4) + ridx, scol]
            for c2 in range(2):
                xg2[:, :, h, c2, :] = xs[:, 2 * (8 + c2) + ridx, scol]
        m = {
            "a_w": a_host,
            "xbm": np.ascontiguousarray(xbm.reshape(NMID + 1, 128, 16 * TN)),
            "xg2": np.ascontiguousarray(xg2.reshape(NMID + 1, 128, 4 * TN)),
            "x4": np.ascontiguousarray(np.tile(xs, (1, 4, 1))),
            "rep": rep_host,
            "x0": np.ascontiguousarray(x0_all[:, sl]),
            "pf": pf_host,
            "pl": pl_host,
            "fscale": fscale_host,
        }
        in_maps.append(m)

    res = bass_utils.run_bass_kernel_spmd(nc, in_maps, core_ids=list(range(N_CORES)))
    _cached["in_maps"] = in_maps

    out = np.empty((B, O), np.float32)
    for ci in range(N_CORES):
        out[ci * BL:(ci + 1) * BL, :] = res.results[ci]["out"].T
    return out


# revision 23
# speedup vs baseline: 1.0935x; 1.0935x over previous
"""Trainium2 Bass kernel for the CMPO2/GTN MPS-chain contraction (v6).

Computation (see harness reference): for each sample s,
    v0  = psi_first^T x[s,0]                                  [D]
    v_{i+1}[e] = sum_{d,p} v_i[d] psi_mid[i][d,e,p] x[s,1+i,p]   (62 steps)
    out_vec[s] = sum_{d,p} v[d] psi_last[d,p,o] x[s,63,p]     [B, O]
    out[s] = c * out_vec[s]   with c the (batch-independent) phi-chain scalar.

Mapping: batch-parallel over 8 cores (1024 samples/core).  Each middle step is
    u[(p,d), s] = v[d, s] * x[s, p]     (DVE fp16 2x elementwise)
    v_new       = A_flat^T @ u          (PE, K=2048 in 16 chunks of 128)
with the [vT; vT] stationary-duplication trick giving the 128-row v operand
for free from PSUM.

v6 structure (why it is shaped this way):
  * The per-step serial chain (PSUM evac -> v2 -> DVE muls -> matmuls) is
    ~12us and cannot be overlapped within one chain.  The batch is split into
    TWO independent half-chains (512 samples each) interleaved half a step
    out of phase, so the kernel is engine-throughput-bound (~10us/step, the
    DVE fp16 2x multiply floor) instead of chain-latency-bound.
  * 6 of 16 x-broadcast chunks are produced on-chip by exact 0/1-stationary
    replication matmuls (K=32, row-tiled on distinct PE array quadrants so
    pair-tiles run concurrently), the other 10 streamed pre-broadcast from
    HBM.  Cuts HBM from ~4.5MB/step to ~3.3MB/step, below the DVE floor.
  * Broadcast matmuls + their PSUM evacs are interleaved INTO the main
    matmul stream at three points (spaced so PSUM slot recycling never
    blocks the in-order PE queue) and produce tiles for step i+1, keeping
    them entirely off the critical chain.
  * phi chain scalar is computed on the host (batch-independent) and folded
    into the final output scale; everything streams prefetched 2 steps out.
"""

import numpy as np

N_CORES = 8
B, Q, P, D, L, O = 8192, 64, 32, 64, 64, 10
BL = B // N_CORES          # batch per core
TN = 512                   # matmul free-dim tile == half-batch
NCH = (D * P) // 128       # 16 K-chunks of 128 over (p,d)
NMID = L - 2               # 62 middle sites
SH_LAST = 6                # 2^SH_LAST folded into psi_last
VBAND = 16.0               # target |v| band for the scale schedule

NB = 6                     # chunks 10..15 produced via PE broadcast
NST = NCH - NB             # 10 streamed chunks
ROW_TILED = True

# global row r in 0..2047 of u/(A rows): p = 2*(r//128) + (r%128)//64 ; d = r%64
_P_IDX = np.repeat(np.arange(P), D)          # [2048]
_D_IDX = np.tile(np.arange(D), P)            # [2048]

_cached = {}


def _ensure_path():
    import sys
    for p in ("/opt/trn_rl_repo", "/root/.axon_site/_ro/trn_rl_repo"):
        try:
            import concourse  # noqa: F401
            return
        except Exception:
            if p not in sys.path:
                sys.path.insert(0, p)
    import concourse  # noqa: F401


def _build_program():
    """Build + compile the Bass/Tile program (shared by all 8 cores)."""
    _ensure_path()
    from concourse import bacc, tile, mybir

    dt = mybir.dt
    nc = bacc.Bacc(
        "TRN2",
        target_bir_lowering=False,
        debug=False,
        enable_asserts=False,
        num_devices=N_CORES,
    )

    # xbm: streamed chunks 0..7, layout [i, r, (h2, g2, c4, s)]
    # xg2: streamed part of the mixed group (chunks 8, 9): [i, r, (h2, c2, s)]
    a_d = nc.dram_tensor("a_w", [NMID, 128, NCH * 128], dt.float16, kind="ExternalInput").ap()
    xbm_d = nc.dram_tensor("xbm", [NMID + 1, 128, 22 * TN], dt.float16, kind="ExternalInput").ap()
    rep_d = nc.dram_tensor("rep", [128, 2 * 128], dt.float16, kind="ExternalInput").ap()
    x0_d = nc.dram_tensor("x0", [P, BL], dt.float16, kind="ExternalInput").ap()
    pf_d = nc.dram_tensor("pf", [P, 128], dt.float16, kind="ExternalInput").ap()
    pl_d = nc.dram_tensor("pl", [128, NCH * O], dt.float16, kind="ExternalInput").ap()
    sc_d = nc.dram_tensor("fscale", [1, 1], dt.float32, kind="ExternalInput").ap()
    out_d = nc.dram_tensor("out", [O, BL], dt.float32, kind="ExternalOutput").ap()

    with tile.TileContext(nc) as tc:
        with tc.tile_pool(name="const", bufs=1) as cpool, \
             tc.tile_pool(name="aw", bufs=3) as apool, \
             tc.tile_pool(name="xbp", bufs=2) as xbpool, \
             tc.tile_pool(name="bcg", bufs=6) as bcgpool, \
             tc.tile_pool(name="vrp", bufs=4) as vrpool, \
             tc.tile_pool(name="up", bufs=6) as upool, \
             tc.tile_pool(name="misc", bufs=1) as mpool, \
             tc.tile_pool(name="pvp", bufs=4, space="PSUM") as pvpool, \
             tc.tile_pool(name="bcp", bufs=2, space="PSUM") as bcpool:

            pf_sb = cpool.tile([P, 128], dt.float16, name="pf_sb")
            nc.sync.dma_start(out=pf_sb, in_=pf_d)
            pl_sb = cpool.tile([128, NCH * O], dt.float16, name="pl_sb")
            nc.sync.dma_start(out=pl_sb, in_=pl_d)
            x0_sb = cpool.tile([P, BL], dt.float16, name="x0_sb")
            nc.sync.dma_start(out=x0_sb, in_=x0_d)
            rep_sb = cpool.tile([128, 2 * 128], dt.float16, name="rep_sb")
            nc.sync.dma_start(out=rep_sb, in_=rep_d)
            sc_sb = mpool.tile([1, 1], dt.float32, name="sc_sb")
            nc.sync.dma_start(out=sc_sb, in_=sc_d)
            sc10 = mpool.tile([O, 1], dt.float32, name="sc10")
            nc.gpsimd.partition_broadcast(sc10, sc_sb)

            def emit_loads(i):
                """Prefetch step i's streamed tiles in TWO large DMAs.
                xbm layout: [h0g0|h0g1|h1g0|h1g1 (16TN) | x4 (2TN) |
                xg2 (h2, c2, s) (4TN)].  Returns (xbm, [bcE_h0, bcE_h1],
                a_raw); x4 and xg2 are views into xbm."""
                xbm = xbpool.tile([128, 22 * TN], dt.float16, name="xbm")
                half = 11 * TN
                nc.sync.dma_start(out=xbm[:, :half], in_=xbm_d[i, :, :half])
                nc.scalar.dma_start(out=xbm[:, half:], in_=xbm_d[i, :, half:])
                bce = []
                for h in range(2):
                    e = bcgpool.tile([128, 6 * TN], dt.float16, name="bcE")
                    bce.append(e)
                a_raw = None
                if i < NMID:
                    a_raw = apool.tile([128, NCH * 128], dt.float16, name="a_raw")
                    nc.scalar.dma_start(out=a_raw, in_=a_d[i])
                return xbm, bce, a_raw

            def emit_bcast_pair(jp, xbm, bce, h):
                """PSUM pair-tile for bcast chunks (10+2jp, 11+2jp), half h:
                two row-tiled matmuls on distinct quadrants, one scalar evac
                into bcE_h columns [jp*2TN : (jp+1)*2TN]."""
                x4v = xbm[:, 16 * TN:18 * TN]
                bps = bcpool.tile([128, 2 * TN], dt.float32, name="bps")
                for u in range(2):
                    j = 2 * jp + u
                    k = (j % 4) if ROW_TILED else 0
                    jq = (j // 4) if ROW_TILED else 0
                    nc.tensor.matmul(
                        out=bps[:, u * TN:(u + 1) * TN],
                        lhsT=rep_sb[32 * k:32 * (k + 1), jq * 128:(jq + 1) * 128],
                        rhs=x4v[32 * k:32 * (k + 1), h * TN:(h + 1) * TN],
                        start=True, stop=True,
                        tile_position=(32 * k, 0) if ROW_TILED else None)
                nc.scalar.copy(out=bce[h][:, jp * 2 * TN:(jp + 1) * 2 * TN],
                               in_=bps)

            # --- v0 per half ---
            pv_cur = []
            for h in range(2):
                pv = pvpool.tile([128, TN], dt.float32, name="pv")
                nc.tensor.matmul(out=pv, lhsT=pf_sb,
                                 rhs=x0_sb[:, h * TN:(h + 1) * TN],
                                 start=True, stop=True)
                pv_cur.append(pv)

            tiles_cur = emit_loads(0)
            for h in range(2):                  # boot: bcast tiles for step 0
                for jp in range(3):
                    emit_bcast_pair(jp, tiles_cur[0], tiles_cur[1], h)
            tiles_nxt = emit_loads(1)

            for i in range(NMID + 1):
                last = (i == NMID)
                xbm, bce, a_raw = tiles_cur
                a_lhs = None if last else a_raw.rearrange("p (c m) -> p c m", c=NCH)

                pv_nxt = []
                for h in range(2):
                    pv = pvpool.tile([O if last else 128, TN], dt.float32, name="pv")
                    pv_nxt.append(pv)

                # v-dup builds for BOTH halves first: evac (ACT) + ladder (DVE)
                v4s, uss = [], []
                for h in range(2):
                    v4 = vrpool.tile([128, 4 * TN], dt.float16, name="v4")
                    nc.scalar.copy(out=v4[:, 0:TN], in_=pv_cur[h])
                    nc.vector.tensor_copy(v4[:, TN:2 * TN], v4[:, 0:TN])
                    nc.vector.tensor_copy(v4[:, 2 * TN:4 * TN], v4[:, 0:2 * TN])
                    v4s.append(v4)

                # u products, both halves back-to-back on the DVE:
                # g0, g1 streamed (FD2048); g2a streamed chunks 8,9 (FD1024);
                # g2b bcast chunks 10,11 (FD1024); g3 bcast 12..15 (FD2048)
                for h in range(2):
                    v4 = v4s[h]
                    us = []
                    for g in range(2):
                        u4 = upool.tile([128, 4 * TN], dt.float16, name="u4")
                        xin = xbm[:, (h * 2 + g) * 4 * TN:(h * 2 + g + 1) * 4 * TN]
                        nc.vector.tensor_mul(u4, v4, xin)
                        us.append(u4)
                    u2a = upool.tile([128, 2 * TN], dt.float16, name="u2a")
                    nc.vector.tensor_mul(
                        u2a, v4[:, 0:2 * TN],
                        xbm[:, (18 + 2 * h) * TN:(20 + 2 * h) * TN])
                    us.append(u2a)
                    u2b = upool.tile([128, 2 * TN], dt.float16, name="u2a")
                    nc.vector.tensor_mul(u2b, v4[:, 0:2 * TN], bce[h][:, 0:2 * TN])
                    us.append(u2b)
                    u4c = upool.tile([128, 4 * TN], dt.float16, name="u4")
                    nc.vector.tensor_mul(u4c, v4, bce[h][:, 2 * TN:6 * TN])
                    us.append(u4c)
                    uss.append(us)

                tiles_fut = emit_loads(i + 2) if i + 2 <= NMID else None

                # main matmuls per half; next step's bcast pair-tiles
                # interleave into half-0's stream
                for h in range(2):
                    us = uss[h]
                    for c in range(NCH):
                        if c < 8:
                            rhs = us[c // 4][:, (c % 4) * TN:(c % 4 + 1) * TN]
                        elif c < 10:
                            rhs = us[2][:, (c - 8) * TN:(c - 7) * TN]
                        elif c < 12:
                            rhs = us[3][:, (c - 10) * TN:(c - 9) * TN]
                        else:
                            rhs = us[4][:, (c - 12) * TN:(c - 11) * TN]
                        lhs = pl_sb[:, c * O:(c + 1) * O] if last else a_lhs[:, c]
                        nc.tensor.matmul(
                            out=pv_nxt[h],
                            lhsT=lhs,
                            rhs=rhs,
                            start=(c == 0), stop=(c == NCH - 1))
                        if not last and h == 0 and c in (5, 10, 15):
                            jp = {5: 0, 10: 1, 15: 2}[c]
                            emit_bcast_pair(jp, tiles_nxt[0], tiles_nxt[1], 0)
                            emit_bcast_pair(jp, tiles_nxt[0], tiles_nxt[1], 1)

                pv_cur = pv_nxt
                if not last:
                    tiles_cur = tiles_nxt
                    tiles_nxt = tiles_fut

            # --- final scale by host-folded phi scalar ---
            out_sb = mpool.tile([O, BL], dt.float32, name="out_sb")
            for h in range(2):
                nc.scalar.mul(out=out_sb[:, h * TN:(h + 1) * TN],
                              in_=pv_cur[h], mul=sc10)
            nc.sync.dma_start(out=out_d, in_=out_sb)

    nc.compile()
    return nc


def _scale_schedule(x, psi_first, psi_mid, nsub=128):
    """Static per-step power-of-2 downscales keeping |v| in a small band."""
    xs = np.asarray(x[:nsub], np.float32)
    v = xs[:, 0] @ np.asarray(psi_first, np.float32).T
    ks = []
    for i in range(NMID):
        A = np.asarray(psi_mid[i], np.float32)            # [d, e, p]
        xi = xs[:, 1 + i]                                  # [s, p]
        u = np.einsum('sd,sp->sdp', v, xi).reshape(nsub, D * P)
        v = u @ A.transpose(0, 2, 1).reshape(D * P, D)
        vm = float(np.abs(v).max())
        k = 0
        while vm * 2.0 ** (-k) > VBAND:
            k += 1
        ks.append(k)
        v = v * 2.0 ** (-k)
    return ks


def kernel(x, psi_first, psi_mid, psi_last, phi_first, phi_mid, phi_last):
    _ensure_path()
    from concourse import bass_utils

    f16 = np.float16
    x = np.asarray(x, np.float32)
    psi_first = np.asarray(psi_first, np.float32)
    psi_mid = np.asarray(psi_mid, np.float32)
    psi_last = np.asarray(psi_last, np.float32)
    phi_first = np.asarray(phi_first, np.float64)
    phi_mid = np.asarray(phi_mid, np.float64)
    phi_last = np.asarray(phi_last, np.float64)

    if "nc" not in _cached:
        _cached["nc"] = _build_program()
    nc = _cached["nc"]

    ks = _scale_schedule(x, psi_first, psi_mid)

    # host phi chain: batch-independent scalar c
    w = phi_first[:, 0].copy()
    for i in range(NMID):
        w = w @ phi_mid[i, :, :, 1 + i]
    c_phi = float(w @ phi_last[:, Q - 1])

    scales = (2.0 ** -np.asarray(ks, np.float64)).astype(np.float32)
    # A2[i, r, e] = psi_mid[i, d(r), e, p(r)] * s_i  -> [62, 2048, 64]
    A2 = psi_mid.transpose(0, 1, 3, 2)[:, _D_IDX, _P_IDX, :]
    A2 = A2 * scales[:, None, None]
    A2c = A2.reshape(NMID, NCH, 128, D)
    # pre-duplicated stationary columns: [i, r, c*128 + j*64 + e]
    A2d = np.broadcast_to(A2c[:, :, :, None, :], (NMID, NCH, 128, 2, D))
    a_host = np.ascontiguousarray(
        A2d.transpose(0, 2, 1, 3, 4).reshape(NMID, 128, NCH * 128)
    ).astype(f16)

    pf_host = np.concatenate([psi_first.T, psi_first.T], axis=1).astype(f16)

    pl2 = (psi_last * (2.0 ** SH_LAST))[_D_IDX, _P_IDX, :]
    pl_host = np.ascontiguousarray(
        pl2.reshape(NCH, 128, O).transpose(1, 0, 2).reshape(128, NCH * O)
    ).astype(f16)

    fscale_host = np.array(
        [[c_phi * 2.0 ** (sum(ks) - SH_LAST)]], dtype=np.float32)

    # rep[32k + p, jq*128 + m] = 1 if p == 2*c + m//64 (c = 10 + j)
    rep_host = np.zeros((128, 2 * 128), f16)
    for j in range(NB):
        c = NST + j
        k = (j % 4) if ROW_TILED else 0
        jq = (j // 4) if ROW_TILED else 0
        for m in range(128):
            p = 2 * c + m // 64
            rep_host[32 * k + p, jq * 128 + m] = 1.0

    # per-core batch shards
    xt = x.transpose(1, 2, 0).astype(f16)         # [Q, P, B]
    x0_all = xt[0]
    ridx = np.arange(128) // 64
    in_maps = []
    for ci in range(N_CORES):
        sl = slice(ci * BL, (ci + 1) * BL)
        xs = np.ascontiguousarray(xt[1:, :, sl])            # [63, P, BL]
        xbm = np.empty((NMID + 1, 128, 22 * TN), f16)
        xbv = xbm[:, :, 0:16 * TN].reshape(NMID + 1, 128, 2, 2, 4, TN)
        xg2 = xbm[:, :, 18 * TN:22 * TN].reshape(NMID + 1, 128, 2, 2, TN)
        for h in range(2):
            scol = slice(h * TN, (h + 1) * TN)
            for g in range(2):
                for c4 in range(4):
                    xbv[:, :, h, g, c4, :] = xs[:, 2 * (4 * g + c4) + ridx, scol]
            for c2 in range(2):
                xg2[:, :, h, c2, :] = xs[:, 2 * (8 + c2) + ridx, scol]
        xbm[:, :, 16 * TN:18 * TN] = np.tile(xs, (1, 4, 1))
        m = {
            "a_w": a_host,
            "xbm": xbm,
            "rep": rep_host,
            "x0": np.ascontiguousarray(x0_all[:, sl]),
            "pf": pf_host,
            "pl": pl_host,
            "fscale": fscale_host,
        }
        in_maps.append(m)

    res = bass_utils.run_bass_kernel_spmd(nc, in_maps, core_ids=list(range(N_CORES)))
    _cached["in_maps"] = in_maps

    out = np.empty((B, O), np.float32)
    for ci in range(N_CORES):
        out[ci * BL:(ci + 1) * BL, :] = res.results[ci]["out"].T
    return out


# revision 24
# speedup vs baseline: 1.2817x; 1.1721x over previous
"""Trainium2 Bass kernel for the CMPO2/GTN MPS-chain contraction.

Computation (see harness reference): for each sample s,
    v0  = psi_first^T x[s,0]                                  [D]
    v_{i+1}[e] = sum_{d,p} v_i[d] psi_mid[i][d,e,p] x[s,1+i,p]   (62 steps)
    out_vec[s] = sum_{d,p} v[d] psi_last[d,p,:] x[s,63,p]     [O]
    out[s] = c * out_vec[s]   with c the (batch-independent) phi-chain scalar.

Strategy: data-parallel over batch across 8 cores (1024 samples/core),
MPS parameters replicated.  Per middle step the contraction is mapped as
    u[s,(p,d)] = v[s,d] * x[s,p]        (outer product, fp16, p-major rows)
    v_new      = u @ A_flat             (PE matmul, K=2048 in 16 chunks of 128)
The per-sample v broadcast is obtained for free: the A stationaries are
column-duplicated so each accumulation chain outputs [vT; vT] on all 128
PSUM partitions, and the x-side partition broadcast (which is input data,
not dependent on v) is precomputed on the host and streamed from HBM by
the otherwise-idle DMA engines.  The vector engine does the outer products
(fp16 2x mode); the scalar engine only evacuates the small [vT; vT].
fp16 overflow is prevented by folding static power-of-2 scales (derived
from a host-side subsample) into the A weights; the inverse scale is
folded into the phi-chain scalar, computed on-device in fp32.
"""

import numpy as np

N_CORES = 8
B, Q, P, D, L, O = 8192, 64, 32, 64, 64, 10
BL = B // N_CORES          # batch per core
TN = 512                   # matmul free-dim tile (one PSUM bank of fp32)
NT = BL // TN              # N tiles per batch shard
NCH = (D * P) // 128       # 16 K-chunks of 128 over (p,d)
NG = 8                     # chunk pairs (2 chunks each) for paired DVE muls
NMID = L - 2               # 62 middle sites
SH_LAST = 6                # 2^SH_LAST folded into psi_last (fp16 subnormal avoidance)
VBAND = 16.0               # target |v| band for the scale schedule

# global row r in 0..2047 of u/(A rows): p = 2*(r//128) + (r%128)//64 ; d = r%64
_P_IDX = np.repeat(np.arange(P), D)          # [2048]
_D_IDX = np.tile(np.arange(D), P)            # [2048]

_cached = {}


def _ensure_path():
    import sys
    for p in ("/opt/trn_rl_repo", "/root/.axon_site/_ro/trn_rl_repo"):
        try:
            import concourse  # noqa: F401
            return
        except Exception:
            if p not in sys.path:
                sys.path.insert(0, p)
    import concourse  # noqa: F401


def _build_program():
    """Build + compile the Bass/Tile program (shared by all 8 cores)."""
    _ensure_path()
    from concourse import bacc, tile, mybir

    dt = mybir.dt
    nc = bacc.Bacc(
        "TRN2",
        target_bir_lowering=False,
        debug=False,
        enable_asserts=False,
        num_devices=N_CORES,
    )

    a_d = nc.dram_tensor("a_w", [NMID, 128, NCH * D], dt.float16, kind="ExternalInput").ap()
    xb_d = nc.dram_tensor("xb", [NMID + 1, 2, 128, 8 * BL], dt.float16, kind="ExternalInput").ap()
    x0_d = nc.dram_tensor("x0", [P, BL], dt.float16, kind="ExternalInput").ap()
    pf_d = nc.dram_tensor("pf", [P, 128], dt.float16, kind="ExternalInput").ap()
    pl_d = nc.dram_tensor("pl", [128, NCH * O], dt.float16, kind="ExternalInput").ap()
    pm_d = nc.dram_tensor("phim", [D, NMID * D], dt.float32, kind="ExternalInput").ap()
    w0_d = nc.dram_tensor("w0", [D, 1], dt.float32, kind="ExternalInput").ap()
    plc_d = nc.dram_tensor("phil", [D, 1], dt.float32, kind="ExternalInput").ap()
    out_d = nc.dram_tensor("out", [O, BL], dt.float32, kind="ExternalOutput").ap()

    with tile.TileContext(nc) as tc:
        with tc.tile_pool(name="const", bufs=1) as cpool, \
             tc.tile_pool(name="aw", bufs=3) as apool, \
             tc.tile_pool(name="awd", bufs=2) as adpool, \
             tc.tile_pool(name="xbp", bufs=6) as xbpool, \
             tc.tile_pool(name="vrp", bufs=2) as vrpool, \
             tc.tile_pool(name="up", bufs=10) as upool, \
             tc.tile_pool(name="misc", bufs=1) as mpool, \
             tc.tile_pool(name="wp", bufs=2) as wpool, \
             tc.tile_pool(name="pvp", bufs=4, space="PSUM") as pvpool, \
             tc.tile_pool(name="phpp", bufs=2, space="PSUM") as phpool:

            # --- constants / per-core inputs resident in SBUF ---
            pf_sb = cpool.tile([P, 128], dt.float16, name="pf_sb")
            nc.sync.dma_start(out=pf_sb, in_=pf_d)
            pl_sb = cpool.tile([128, NCH * O], dt.float16, name="pl_sb")
            nc.sync.dma_start(out=pl_sb, in_=pl_d)
            pm_sb = cpool.tile([D, NMID * D], dt.float32, name="pm_sb")
            nc.sync.dma_start(out=pm_sb, in_=pm_d)
            plc_sb = cpool.tile([D, 1], dt.float32, name="plc_sb")
            nc.sync.dma_start(out=plc_sb, in_=plc_d)
            x0_sb = cpool.tile([P, BL], dt.float16, name="x0_sb")
            nc.sync.dma_start(out=x0_sb, in_=x0_d)

            w_cur = wpool.tile([D, 1], dt.float32, name="wv")
            nc.sync.dma_start(out=w_cur, in_=w0_d)

            # --- v0 = [psi_first^T | psi_first^T] @ x0 -> [v0 ; v0] ---
            pv_cur = []
            for t in range(NT):
                pv = pvpool.tile([128, TN], dt.float32, name="pv")
                nc.tensor.matmul(out=pv, lhsT=pf_sb,
                                 rhs=x0_sb[:, t * TN:(t + 1) * TN],
                                 start=True, stop=True)
                pv_cur.append(pv)

            po = None
            for i in range(NMID + 1):
                last = (i == NMID)
                # evacuate [vT; vT] into v2[:, 0:BL] per N-tile half, each
                # followed by its own dup copy, so each half's outer products
                # and matmuls can proceed while the other half is still in
                # flight (keeps the PE warm and busy across the step chain).
                v2 = vrpool.tile([128, 2 * BL], dt.float16, name="v2")
                for t in range(NT):
                    nc.scalar.copy(out=v2[:, t * TN:(t + 1) * TN], in_=pv_cur[t])
                    nc.vector.tensor_copy(v2[:, BL + t * TN:BL + (t + 1) * TN],
                                          v2[:, t * TN:(t + 1) * TN])

                # x-side broadcast from HBM as two 2MB transfers (best
                # DMA efficiency); pair tiles are column slices of them.
                xbq = []
                for gq in range(2):
                    xq_sb = xbpool.tile([128, 8 * BL], dt.float16, name="xq_sb")
                    eng = nc.sync if gq == 0 else nc.scalar
                    eng.dma_start(out=xq_sb, in_=xb_d[i, gq])
                    xbq.append(xq_sb)
                xbs = [xbq[g // 4][:, (g % 4) * 2 * BL:(g % 4 + 1) * 2 * BL]
                       for g in range(NG)]

                if not last:
                    # stream the un-duplicated A chunk-columns, duplicate
                    # on-chip (two strided copies on the idle scalar engine)
                    a_raw = apool.tile([128, NCH * D], dt.float16, name="a_raw")
                    nc.gpsimd.dma_start(out=a_raw, in_=a_d[i])
                    a_sb = adpool.tile([128, NCH * 128], dt.float16, name="a_sb")
                    av = a_sb.rearrange("p (c j e) -> p c j e", c=NCH, j=2, e=D)
                    ar = a_raw.rearrange("p (c e) -> p c e", c=NCH)
                    nc.scalar.copy(out=av[:, :, 0, :], in_=ar)
                    nc.scalar.copy(out=av[:, :, 1, :], in_=ar)
                    pv_nxt = []
                    for t in range(NT):
                        pv = pvpool.tile([128, TN], dt.float32, name="pv")
                        pv_nxt.append(pv)
                else:
                    po = []
                    for t in range(NT):
                        p_o = pvpool.tile([O, TN], dt.float32, name="pv")
                        po.append(p_o)

                # outer products and matmuls, emitted per N-tile half so the
                # two halves software-pipeline: while the DVE produces half
                # t1's u tiles, the PE consumes half t0's.  Columns of
                # v2/xb/u tiles are laid out (q, t, s) with q the chunk
                # within the pair, so the t-half of a pair is a strided view.
                us = []
                for g in range(NG):
                    u2 = upool.tile([128, 2 * BL], dt.float16, name="u2")
                    us.append(u2)
                lhs_sb = pl_sb if last else a_sb
                lhs_w = O if last else 128
                out_ps = po if last else pv_nxt
                for t in range(NT):
                    for q in range(2):
                        sl = slice(q * BL + t * TN, q * BL + (t + 1) * TN)
                        nc.vector.tensor_mul(us[0][:, sl], v2[:, t * TN:(t + 1) * TN],
                                             xbs[0][:, sl])
                    for g in range(1, NG):
                        v2t = v2.rearrange("p (q t s) -> p q t s", q=2, t=NT, s=TN)[:, :, t, :]
                        xbt = xbs[g].rearrange("p (q t s) -> p q t s", q=2, t=NT, s=TN)[:, :, t, :]
                        ut = us[g].rearrange("p (q t s) -> p q t s", q=2, t=NT, s=TN)[:, :, t, :]
                        nc.vector.tensor_mul(ut, v2t, xbt)
                    for c in range(NCH):
                        g, q = c // 2, c % 2
                        sl = slice(q * BL + t * TN, q * BL + (t + 1) * TN)
                        nc.tensor.matmul(
                            out=out_ps[t],
                            lhsT=lhs_sb[:, c * lhs_w:(c + 1) * lhs_w],
                            rhs=us[g][:, sl],
                            start=(c == 0), stop=(c == NCH - 1))
                if not last:
                    # phi chain matvec, interleaved (PE fp32, tiny)
                    php = phpool.tile([D, 1], dt.float32, name="php")
                    nc.tensor.matmul(out=php, lhsT=pm_sb[:, i * D:(i + 1) * D],
                                     rhs=w_cur, start=True, stop=True)
                    w_nxt = wpool.tile([D, 1], dt.float32, name="wv")
                    nc.scalar.copy(out=w_nxt, in_=php)
                    w_cur = w_nxt
                    pv_cur = pv_nxt

            # --- c = w^T phi_last' ; broadcast to O partitions; scale output ---
            cps = phpool.tile([1, 1], dt.float32, name="php")
            nc.tensor.matmul(out=cps, lhsT=plc_sb, rhs=w_cur, start=True, stop=True)
            c_sb = mpool.tile([1, 1], dt.float32, name="c_sb")
            nc.scalar.copy(out=c_sb, in_=cps)
            c10_sb = mpool.tile([O, 1], dt.float32, name="c10_sb")
            nc.gpsimd.partition_broadcast(c10_sb, c_sb)

            out_sb = mpool.tile([O, BL], dt.float32, name="out_sb")
            for t in range(NT):
                nc.scalar.mul(out=out_sb[:, t * TN:(t + 1) * TN], in_=po[t], mul=c10_sb)
            nc.sync.dma_start(out=out_d, in_=out_sb)

    nc.compile()
    return nc


def _scale_schedule(x, psi_first, psi_mid, nsub=128):
    """Static per-step power-of-2 downscales keeping |v| in a small band."""
    xs = np.asarray(x[:nsub], np.float32)
    v = xs[:, 0] @ np.asarray(psi_first, np.float32).T
    ks = []
    for i in range(NMID):
        A = np.asarray(psi_mid[i], np.float32)            # [d, e, p]
        xi = xs[:, 1 + i]                                  # [s, p]
        u = np.einsum('sd,sp->sdp', v, xi).reshape(nsub, D * P)
        v = u @ A.transpose(0, 2, 1).reshape(D * P, D)
        vm = float(np.abs(v).max())
        k = 0
        while vm * 2.0 ** (-k) > VBAND:
            k += 1
        ks.append(k)
        v = v * 2.0 ** (-k)
    return ks


def kernel(x, psi_first, psi_mid, psi_last, phi_first, phi_mid, phi_last):
    _ensure_path()
    from concourse import bass_utils

    f16 = np.float16
    x = np.asarray(x, np.float32)
    psi_first = np.asarray(psi_first, np.float32)
    psi_mid = np.asarray(psi_mid, np.float32)
    psi_last = np.asarray(psi_last, np.float32)
    phi_first = np.asarray(phi_first, np.float32)
    phi_mid = np.asarray(phi_mid, np.float32)
    phi_last = np.asarray(phi_last, np.float32)

    if "nc" not in _cached:
        _cached["nc"] = _build_program()
    nc = _cached["nc"]

    ks = _scale_schedule(x, psi_first, psi_mid)

    # --- shared weight-side arrays (p-major rows, duplicated columns) ---
    scales = (2.0 ** -np.asarray(ks, np.float64)).astype(np.float32)
    # A2[i, r, e] = psi_mid[i, d(r), e, p(r)] * s_i  -> [62, 2048, 64]
    A2 = psi_mid.transpose(0, 1, 3, 2)[:, _D_IDX, _P_IDX, :]        # [62, 2048, 64]
    A2 = A2 * scales[:, None, None]
    A2c = A2.reshape(NMID, NCH, 128, D)
    a_host = np.ascontiguousarray(
        A2c.transpose(0, 2, 1, 3).reshape(NMID, 128, NCH * D)
    ).astype(f16)

    pf_host = np.concatenate([psi_first.T, psi_first.T], axis=1).astype(f16)  # [32, 128]

    # pl2[r, o] = psi_last[d(r), p(r), o] * 2^SH -> chunked [128, 16*O]
    pl2 = (psi_last * (2.0 ** SH_LAST))[_D_IDX, _P_IDX, :]          # [2048, O]
    pl_host = np.ascontiguousarray(
        pl2.reshape(NCH, 128, O).transpose(1, 0, 2).reshape(128, NCH * O)
    ).astype(f16)

    phiM = phi_mid[np.arange(NMID), :, :, np.arange(1, NMID + 1)]   # [62, e, f]
    pm_host = np.ascontiguousarray(phiM.transpose(1, 0, 2).reshape(D, NMID * D)).astype(np.float32)
    w0_host = np.ascontiguousarray(phi_first[:, 0:1]).astype(np.float32)
    plc_host = np.ascontiguousarray(
        phi_last[:, Q - 1:Q] * (2.0 ** (sum(ks) - SH_LAST))
    ).astype(np.float32)

    # --- per-core batch shards: x-side broadcast [63, NG, 128, 4*BL] ---
    xt = x.transpose(1, 2, 0).astype(f16)         # [Q, P, B]
    x0_all = xt[0]                                # [P, B]
    # chunk c rows r(128): p = 2c + r//64 ; group tile [128, 4*BL]:
    #   xb[i, g, r, q*BL + s] = xt[1+i, 2*(4g+q) + r//64, s]
    in_maps = []
    for ci in range(N_CORES):
        sl = slice(ci * BL, (ci + 1) * BL)
        xs = np.ascontiguousarray(xt[1:, :, sl])            # [63, P, BL]
        xb = xs[:, np.repeat(np.arange(P), D), :]           # [63, 2048, BL]
        xb = xb.reshape(NMID + 1, NG, 2, 128, BL).transpose(0, 1, 3, 2, 4)
        xb = xb.reshape(NMID + 1, NG, 128, 2 * BL)
        xb = xb.reshape(NMID + 1, 2, 4, 128, 2 * BL).transpose(0, 1, 3, 2, 4)
        xb = np.ascontiguousarray(xb.reshape(NMID + 1, 2, 128, 8 * BL))
        in_maps.append({
            "a_w": a_host,
            "xb": xb,
            "x0": np.ascontiguousarray(x0_all[:, sl]),
            "pf": pf_host,
            "pl": pl_host,
            "phim": pm_host,
            "w0": w0_host,
            "phil": plc_host,
        })

    res = bass_utils.run_bass_kernel_spmd(nc, in_maps, core_ids=list(range(N_CORES)))
    _cached["in_maps"] = in_maps

    out = np.empty((B, O), np.float32)
    for ci in range(N_CORES):
        out[ci * BL:(ci + 1) * BL, :] = res.results[ci]["out"].T
    return out



# revision 25
# speedup vs baseline: 1.4861x; 1.1595x over previous
"""Trainium2 Bass kernel for the CMPO2/GTN MPS-chain contraction.

Computation (see harness reference): for each sample s,
    v0  = psi_first^T x[s,0]                                  [D]
    v_{i+1}[e] = sum_{d,p} v_i[d] psi_mid[i][d,e,p] x[s,1+i,p]   (62 steps)
    out_vec[s] = sum_{d,p} v[d] psi_last[d,p,:] x[s,63,p]     [O]
    out[s] = c * out_vec[s]   with c the (batch-independent) phi-chain scalar.

Strategy: data-parallel over batch across 8 cores (1024 samples/core),
MPS parameters replicated.  Per middle step the contraction is mapped as
    u[s,(p,d)] = v[s,d] * x[s,p]        (outer product, fp16, p-major rows)
    v_new      = u @ A_flat             (PE matmul, K=2048 in 16 chunks of 128)
The per-sample v broadcast is obtained for free: the A stationaries are
column-duplicated so each accumulation chain outputs [vT; vT] on all 128
PSUM partitions, and the x-side partition broadcast (which is input data,
not dependent on v) is precomputed on the host and streamed from HBM by
the otherwise-idle DMA engines.  The vector engine does the outer products
(fp16 2x mode); the scalar engine only evacuates the small [vT; vT].
fp16 overflow is prevented by folding static power-of-2 scales (derived
from a host-side subsample) into the A weights; the inverse scale is
folded into the phi-chain scalar, computed on-device in fp32.
"""

import numpy as np

N_CORES = 8
B, Q, P, D, L, O = 8192, 64, 32, 64, 64, 10
BL = B // N_CORES          # batch per core
TN = 512                   # matmul free-dim tile (one PSUM bank of fp32)
NT = BL // TN              # N tiles per batch shard
NCH = (D * P) // 128       # 16 K-chunks of 128 over (p,d)
NG = 8                     # chunk pairs (2 chunks each) for paired DVE muls
NMID = L - 2               # 62 middle sites
SH_LAST = 6                # 2^SH_LAST folded into psi_last (fp16 subnormal avoidance)
VBAND = 16.0               # target |v| band for the scale schedule

# global row r in 0..2047 of u/(A rows): p = 2*(r//128) + (r%128)//64 ; d = r%64
_P_IDX = np.repeat(np.arange(P), D)          # [2048]
_D_IDX = np.tile(np.arange(D), P)            # [2048]

_cached = {}


def _ensure_path():
    import sys
    for p in ("/opt/trn_rl_repo", "/root/.axon_site/_ro/trn_rl_repo"):
        try:
            import concourse  # noqa: F401
            return
        except Exception:
            if p not in sys.path:
                sys.path.insert(0, p)
    import concourse  # noqa: F401


def _build_program():
    """Build + compile the Bass/Tile program (shared by all 8 cores)."""
    _ensure_path()
    from concourse import bacc, tile, mybir

    dt = mybir.dt
    nc = bacc.Bacc(
        "TRN2",
        target_bir_lowering=False,
        debug=False,
        enable_asserts=False,
        num_devices=N_CORES,
    )

    a_d = nc.dram_tensor("a_w", [NMID, 128, NCH * D], dt.float16, kind="ExternalInput").ap()
    xb_d = nc.dram_tensor("xb", [NMID + 1, 128, 14 * BL], dt.float16, kind="ExternalInput").ap()
    xs_d = nc.dram_tensor("xs", [NMID + 1, P, BL], dt.float16, kind="ExternalInput").ap()
    rep_d = nc.dram_tensor("rep", [P, 2 * 128], dt.float16, kind="ExternalInput").ap()
    x0_d = nc.dram_tensor("x0", [P, BL], dt.float16, kind="ExternalInput").ap()
    pf_d = nc.dram_tensor("pf", [P, 128], dt.float16, kind="ExternalInput").ap()
    pl_d = nc.dram_tensor("pl", [128, NCH * O], dt.float16, kind="ExternalInput").ap()
    pm_d = nc.dram_tensor("phim", [D, NMID * D], dt.float32, kind="ExternalInput").ap()
    w0_d = nc.dram_tensor("w0", [D, 1], dt.float32, kind="ExternalInput").ap()
    plc_d = nc.dram_tensor("phil", [D, 1], dt.float32, kind="ExternalInput").ap()
    out_d = nc.dram_tensor("out", [O, BL], dt.float32, kind="ExternalOutput").ap()

    with tile.TileContext(nc) as tc:
        with tc.tile_pool(name="const", bufs=1) as cpool, \
             tc.tile_pool(name="aw", bufs=3) as apool, \
             tc.tile_pool(name="awd", bufs=2) as adpool, \
             tc.tile_pool(name="xbp", bufs=3) as xbpool, \
             tc.tile_pool(name="bcx", bufs=2) as bcxpool, \
             tc.tile_pool(name="xsp", bufs=2) as xspool, \
             tc.tile_pool(name="vrp", bufs=2) as vrpool, \
             tc.tile_pool(name="up", bufs=10) as upool, \
             tc.tile_pool(name="misc", bufs=1) as mpool, \
             tc.tile_pool(name="wp", bufs=2) as wpool, \
             tc.tile_pool(name="pvp", bufs=4, space="PSUM") as pvpool, \
             tc.tile_pool(name="phpp", bufs=2, space="PSUM") as phpool, \
             tc.tile_pool(name="bcp", bufs=1, space="PSUM") as bcpool:

            # --- constants / per-core inputs resident in SBUF ---
            pf_sb = cpool.tile([P, 128], dt.float16, name="pf_sb")
            nc.sync.dma_start(out=pf_sb, in_=pf_d)
            pl_sb = cpool.tile([128, NCH * O], dt.float16, name="pl_sb")
            nc.sync.dma_start(out=pl_sb, in_=pl_d)
            pm_sb = cpool.tile([D, NMID * D], dt.float32, name="pm_sb")
            nc.sync.dma_start(out=pm_sb, in_=pm_d)
            plc_sb = cpool.tile([D, 1], dt.float32, name="plc_sb")
            nc.sync.dma_start(out=plc_sb, in_=plc_d)
            x0_sb = cpool.tile([P, BL], dt.float16, name="x0_sb")
            nc.sync.dma_start(out=x0_sb, in_=x0_d)

            rep_sb = cpool.tile([P, 2 * 128], dt.float16, name="rep_sb")
            nc.sync.dma_start(out=rep_sb, in_=rep_d)

            def emit_bcast(i):
                """bcx tile [128, 2*BL] for step i: chunks 14, 15 produced by
                exact 0/1-stationary K=32 matmuls + scalar evacs."""
                xs_sb = xspool.tile([P, BL], dt.float16, name="xs_sb")
                nc.gpsimd.dma_start(out=xs_sb, in_=xs_d[i])
                bcx = bcxpool.tile([128, 2 * BL], dt.float16, name="bcx")
                for j in range(2):
                    bps = bcpool.tile([128, 2 * TN], dt.float32, name="bps")
                    for t in range(NT):
                        nc.tensor.matmul(
                            out=bps[:, t * TN:(t + 1) * TN],
                            lhsT=rep_sb[:, j * 128:(j + 1) * 128],
                            rhs=xs_sb[:, t * TN:(t + 1) * TN],
                            start=True, stop=True)
                    nc.scalar.copy(out=bcx[:, j * BL:(j + 1) * BL], in_=bps)
                return bcx

            w_cur = wpool.tile([D, 1], dt.float32, name="wv")
            nc.sync.dma_start(out=w_cur, in_=w0_d)

            # --- v0 = [psi_first^T | psi_first^T] @ x0 -> [v0 ; v0] ---
            pv_cur = []
            for t in range(NT):
                pv = pvpool.tile([128, TN], dt.float32, name="pv")
                nc.tensor.matmul(out=pv, lhsT=pf_sb,
                                 rhs=x0_sb[:, t * TN:(t + 1) * TN],
                                 start=True, stop=True)
                pv_cur.append(pv)

            bcx_cur = emit_bcast(0)
            po = None
            for i in range(NMID + 1):
                last = (i == NMID)
                # evacuate [vT; vT] into v2[:, 0:BL] per N-tile half, each
                # followed by its own dup copy, so each half's outer products
                # and matmuls can proceed while the other half is still in
                # flight (keeps the PE warm and busy across the step chain).
                v2 = vrpool.tile([128, 2 * BL], dt.float16, name="v2")
                for t in range(NT):
                    nc.scalar.copy(out=v2[:, t * TN:(t + 1) * TN], in_=pv_cur[t])
                    nc.vector.tensor_copy(v2[:, BL + t * TN:BL + (t + 1) * TN],
                                          v2[:, t * TN:(t + 1) * TN])

                # x-side broadcast: 7 groups streamed from HBM (2 DMAs),
                # group 7 (chunks 14, 15) produced on-chip last step.
                xq_sb = xbpool.tile([128, 14 * BL], dt.float16, name="xq_sb")
                nc.sync.dma_start(out=xq_sb[:, :8 * BL], in_=xb_d[i, :, :8 * BL])
                nc.scalar.dma_start(out=xq_sb[:, 8 * BL:], in_=xb_d[i, :, 8 * BL:])
                xbs = [xq_sb[:, g * 2 * BL:(g + 1) * 2 * BL] for g in range(7)]
                xbs.append(bcx_cur)

                if not last:
                    # stream the un-duplicated A chunk-columns, duplicate
                    # on-chip (two strided copies on the idle scalar engine)
                    a_raw = apool.tile([128, NCH * D], dt.float16, name="a_raw")
                    nc.gpsimd.dma_start(out=a_raw, in_=a_d[i])
                    a_sb = adpool.tile([128, NCH * 128], dt.float16, name="a_sb")
                    av = a_sb.rearrange("p (c j e) -> p c j e", c=NCH, j=2, e=D)
                    ar = a_raw.rearrange("p (c e) -> p c e", c=NCH)
                    nc.scalar.copy(out=av[:, :, 0, :], in_=ar)
                    nc.scalar.copy(out=av[:, :, 1, :], in_=ar)
                    pv_nxt = []
                    for t in range(NT):
                        pv = pvpool.tile([128, TN], dt.float32, name="pv")
                        pv_nxt.append(pv)
                else:
                    po = []
                    for t in range(NT):
                        p_o = pvpool.tile([O, TN], dt.float32, name="pv")
                        po.append(p_o)

                # outer products and matmuls, emitted per N-tile half so the
                # two halves software-pipeline: while the DVE produces half
                # t1's u tiles, the PE consumes half t0's.  Columns of
                # v2/xb/u tiles are laid out (q, t, s) with q the chunk
                # within the pair, so the t-half of a pair is a strided view.
                us = []
                for g in range(NG):
                    u2 = upool.tile([128, 2 * BL], dt.float16, name="u2")
                    us.append(u2)
                lhs_sb = pl_sb if last else a_sb
                lhs_w = O if last else 128
                out_ps = po if last else pv_nxt
                for t in range(NT):
                    for q in range(2):
                        sl = slice(q * BL + t * TN, q * BL + (t + 1) * TN)
                        nc.vector.tensor_mul(us[0][:, sl], v2[:, t * TN:(t + 1) * TN],
                                             xbs[0][:, sl])
                    for g in range(1, NG):
                        v2t = v2.rearrange("p (q t s) -> p q t s", q=2, t=NT, s=TN)[:, :, t, :]
                        xbt = xbs[g].rearrange("p (q t s) -> p q t s", q=2, t=NT, s=TN)[:, :, t, :]
                        ut = us[g].rearrange("p (q t s) -> p q t s", q=2, t=NT, s=TN)[:, :, t, :]
                        nc.vector.tensor_mul(ut, v2t, xbt)
                    for c in range(NCH):
                        g, q = c // 2, c % 2
                        sl = slice(q * BL + t * TN, q * BL + (t + 1) * TN)
                        nc.tensor.matmul(
                            out=out_ps[t],
                            lhsT=lhs_sb[:, c * lhs_w:(c + 1) * lhs_w],
                            rhs=us[g][:, sl],
                            start=(c == 0), stop=(c == NCH - 1))
                if not last:
                    # phi chain matvec, interleaved (PE fp32, tiny)
                    php = phpool.tile([D, 1], dt.float32, name="php")
                    nc.tensor.matmul(out=php, lhsT=pm_sb[:, i * D:(i + 1) * D],
                                     rhs=w_cur, start=True, stop=True)
                    w_nxt = wpool.tile([D, 1], dt.float32, name="wv")
                    nc.scalar.copy(out=w_nxt, in_=php)
                    w_cur = w_nxt
                    pv_cur = pv_nxt
                if not last:
                    bcx_cur = emit_bcast(i + 1)

            # --- c = w^T phi_last' ; broadcast to O partitions; scale output ---
            cps = phpool.tile([1, 1], dt.float32, name="php")
            nc.tensor.matmul(out=cps, lhsT=plc_sb, rhs=w_cur, start=True, stop=True)
            c_sb = mpool.tile([1, 1], dt.float32, name="c_sb")
            nc.scalar.copy(out=c_sb, in_=cps)
            c10_sb = mpool.tile([O, 1], dt.float32, name="c10_sb")
            nc.gpsimd.partition_broadcast(c10_sb, c_sb)

            out_sb = mpool.tile([O, BL], dt.float32, name="out_sb")
            for t in range(NT):
                nc.scalar.mul(out=out_sb[:, t * TN:(t + 1) * TN], in_=po[t], mul=c10_sb)
            nc.sync.dma_start(out=out_d, in_=out_sb)

    nc.compile()
    return nc


def _scale_schedule(x, psi_first, psi_mid, nsub=128):
    """Static per-step power-of-2 downscales keeping |v| in a small band."""
    xs = np.asarray(x[:nsub], np.float32)
    v = xs[:, 0] @ np.asarray(psi_first, np.float32).T
    ks = []
    for i in range(NMID):
        A = np.asarray(psi_mid[i], np.float32)            # [d, e, p]
        xi = xs[:, 1 + i]                                  # [s, p]
        u = np.einsum('sd,sp->sdp', v, xi).reshape(nsub, D * P)
        v = u @ A.transpose(0, 2, 1).reshape(D * P, D)
        vm = float(np.abs(v).max())
        k = 0
        while vm * 2.0 ** (-k) > VBAND:
            k += 1
        ks.append(k)
        v = v * 2.0 ** (-k)
    return ks


def kernel(x, psi_first, psi_mid, psi_last, phi_first, phi_mid, phi_last):
    _ensure_path()
    from concourse import bass_utils

    f16 = np.float16
    x = np.asarray(x, np.float32)
    psi_first = np.asarray(psi_first, np.float32)
    psi_mid = np.asarray(psi_mid, np.float32)
    psi_last = np.asarray(psi_last, np.float32)
    phi_first = np.asarray(phi_first, np.float32)
    phi_mid = np.asarray(phi_mid, np.float32)
    phi_last = np.asarray(phi_last, np.float32)

    if "nc" not in _cached:
        _cached["nc"] = _build_program()
    nc = _cached["nc"]

    ks = _scale_schedule(x, psi_first, psi_mid)

    # --- shared weight-side arrays (p-major rows, duplicated columns) ---
    scales = (2.0 ** -np.asarray(ks, np.float64)).astype(np.float32)
    # A2[i, r, e] = psi_mid[i, d(r), e, p(r)] * s_i  -> [62, 2048, 64]
    A2 = psi_mid.transpose(0, 1, 3, 2)[:, _D_IDX, _P_IDX, :]        # [62, 2048, 64]
    A2 = A2 * scales[:, None, None]
    A2c = A2.reshape(NMID, NCH, 128, D)
    a_host = np.ascontiguousarray(
        A2c.transpose(0, 2, 1, 3).reshape(NMID, 128, NCH * D)
    ).astype(f16)

    pf_host = np.concatenate([psi_first.T, psi_first.T], axis=1).astype(f16)  # [32, 128]

    # pl2[r, o] = psi_last[d(r), p(r), o] * 2^SH -> chunked [128, 16*O]
    pl2 = (psi_last * (2.0 ** SH_LAST))[_D_IDX, _P_IDX, :]          # [2048, O]
    pl_host = np.ascontiguousarray(
        pl2.reshape(NCH, 128, O).transpose(1, 0, 2).reshape(128, NCH * O)
    ).astype(f16)

    phiM = phi_mid[np.arange(NMID), :, :, np.arange(1, NMID + 1)]   # [62, e, f]
    pm_host = np.ascontiguousarray(phiM.transpose(1, 0, 2).reshape(D, NMID * D)).astype(np.float32)
    w0_host = np.ascontiguousarray(phi_first[:, 0:1]).astype(np.float32)
    plc_host = np.ascontiguousarray(
        phi_last[:, Q - 1:Q] * (2.0 ** (sum(ks) - SH_LAST))
    ).astype(np.float32)

    # rep[p, j*128 + m] = 1 if p == 2*(14+j) + m//64  (chunks 14, 15)
    rep_host = np.zeros((P, 2 * 128), np.float16)
    for j in range(2):
        for m in range(128):
            rep_host[2 * (14 + j) + m // 64, j * 128 + m] = 1.0

    # --- per-core batch shards: x-side broadcast [63, NG, 128, 4*BL] ---
    xt = x.transpose(1, 2, 0).astype(f16)         # [Q, P, B]
    x0_all = xt[0]                                # [P, B]
    # chunk c rows r(128): p = 2c + r//64 ; group tile [128, 4*BL]:
    #   xb[i, g, r, q*BL + s] = xt[1+i, 2*(4g+q) + r//64, s]
    in_maps = []
    for ci in range(N_CORES):
        sl = slice(ci * BL, (ci + 1) * BL)
        xs = np.ascontiguousarray(xt[1:, :, sl])            # [63, P, BL]
        xb = xs[:, np.repeat(np.arange(P), D), :]           # [63, 2048, BL]
        xb = xb.reshape(NMID + 1, NG, 2, 128, BL).transpose(0, 1, 3, 2, 4)
        xb = xb.reshape(NMID + 1, NG, 128, 2 * BL)
        xb = xb.reshape(NMID + 1, 2, 4, 128, 2 * BL).transpose(0, 1, 3, 2, 4)
        xb = np.ascontiguousarray(xb.reshape(NMID + 1, 2, 128, 8 * BL))
        xbm = np.concatenate([xb[:, 0], xb[:, 1, :, :6 * BL]], axis=2)
        in_maps.append({
            "a_w": a_host,
            "xb": np.ascontiguousarray(xbm),
            "xs": xs,
            "rep": rep_host,
            "x0": np.ascontiguousarray(x0_all[:, sl]),
            "pf": pf_host,
            "pl": pl_host,
            "phim": pm_host,
            "w0": w0_host,
            "phil": plc_host,
        })

    res = bass_utils.run_bass_kernel_spmd(nc, in_maps, core_ids=list(range(N_CORES)))
    _cached["in_maps"] = in_maps

    out = np.empty((B, O), np.float32)
    for ci in range(N_CORES):
        out[ci * BL:(ci + 1) * BL, :] = res.results[ci]["out"].T
    return out

